# revision 27
# baseline (speedup 1.0000x reference)
"""BitLinear (ternary-weight / int8-activation quantized linear) on 8 trn2 NeuronCores.

Math (matches the jax reference up to fp32 rounding):
    eta   = clip(max|x| along k, 1e-5)             per row
    x_q   = round(x * 127 / eta)    in [-127,127]  (round-half-even)
    gamma = clip(mean|w|, 1e-5)                    scalar
    w_q   = round(clip(w / gamma, -1, 1))          in {-1,0,1}
    out   = (x_q @ w_q^T) * (eta/127 * gamma) + bias

x_q / w_q are small integers exactly representable in bf16 and the PE
accumulates in fp32, so the bf16 matmul is EXACT.  Rounding uses the fp32
magic-number trick  rint(t) = (t + 1.5*2^23) - 1.5*2^23  (round-half-even).
The w clip is applied BEFORE scaling:  round(clip(w/g,-1,1)) ==
round(clip(w,-g,g)/g)  (elements |w|>=g map to +-1 either way), saving a pass.
Both quantized operands are PE-transposed as fp32 magic values (t = q + C);
the PSUM->SBUF drain applies -C and converts to bf16 in the same pass.

Sharding: data-parallel over rows of x (16384 -> 2048 rows/core), weight+bias
replicated.

Per-core schedule: the mm is swept in four nb-COLUMNS (512 out-cols each).
Column nb only depends on w tiles 4nb..4nb+3, so column 0 starts ~12us after
gamma while the rest of w is still being quantized.  LDWEIGHTS is fully
hidden behind the previous matmul's stream on trn2, so the 1-LDW-per-MM cost
of column sweeps is zero.

  pass 1  [0,47us]:  stream all 16 w tiles (2 HWDGE rings), |w|-reduce each
                     (DVE); the last 3 tiles stay resident in their pool bufs.
                     x tiles 0-2 ride at the head of the rings.
  gamma   [~50us]:   PE folds the partial sums; gamma/inv_g/-g scalars.
  W-quant [50..180]: per tile: clip (GP), *inv_g+C (ACT), PE fp32-transpose,
                     drain -C -> bf16 wqT (ACT/DVE), paced so column nb's
                     tiles are ready before its sweep begins.
  X       [50..160]: per tile: abs-max reduce (DVE), *qs+C (ACT), PE
                     fp32-transpose, drain -C -> bf16 xqT (ACT/DVE).
  MM      [~64..]:   4 column sweeps x 16 m-blocks; each block: 16 matmuls
                     (kt-accumulate into one PSUM bank), fused dequant+bias
                     (scalar_tensor_tensor: psum*osc + bias_bcast) -> store.
"""

import os
from contextlib import ExitStack

import numpy as np
import ml_dtypes

import concourse.bass as bass
import concourse.bacc as bacc
import concourse.mybir as mybir
import concourse.tile as tile
from concourse.bass_utils import run_bass_kernel_spmd

P = 128
K = 2048
N = 2048
M_CORE = 2048
KT = K // P          # 16
NT = N // P          # 16
MT = M_CORE // P     # 16
NBLK = N // 512      # 4
N_CORES = 8
C_MAGIC = 12582912.0     # 1.5 * 2**23
INV_NK = 1.0 / (N * K)
N_XPRO = 3           # x tiles processed before the mm loops
WSTAGE_BUFS = 4      # pass-1 w pool; tiles 0..3 (emitted last) stay resident

F32 = mybir.dt.float32
BF16 = mybir.dt.bfloat16
ALU = mybir.AluOpType
AXIS = mybir.AxisListType
ACTF = mybir.ActivationFunctionType


def _build_program():
    nc = bacc.Bacc("TRN2", target_bir_lowering=False, debug=False)

    x_d = nc.dram_tensor("x", [M_CORE, K], F32, kind="ExternalInput").ap()
    w_d = nc.dram_tensor("weight", [N, K], F32, kind="ExternalInput").ap()
    b_d = nc.dram_tensor("bias", [P, N], BF16, kind="ExternalInput").ap()
    out_d = nc.dram_tensor("out", [M_CORE, N], F32, kind="ExternalOutput").ap()
    identf_d = nc.inline_tensor(
        np.eye(P, dtype=np.float32), name="ident128f"
    ).ap()
    ident_d = nc.inline_tensor(
        np.eye(P, dtype=ml_dtypes.bfloat16), name="ident128"
    ).ap()

    with tile.TileContext(nc) as tc, ExitStack() as ctx:
        consts = ctx.enter_context(tc.tile_pool(name="consts", bufs=1))
        stats = ctx.enter_context(tc.tile_pool(name="stats", bufs=1))
        bias_p = ctx.enter_context(tc.tile_pool(name="bias_p", bufs=1))
        wqT_p = ctx.enter_context(tc.tile_pool(name="wqT", bufs=1))
        xqT_p = ctx.enter_context(tc.tile_pool(name="xqT", bufs=1))
        wstage = ctx.enter_context(tc.tile_pool(name="wstage", bufs=WSTAGE_BUFS))
        xstage = ctx.enter_context(tc.tile_pool(name="xstage", bufs=3))
        xqst = ctx.enter_context(tc.tile_pool(name="xqst", bufs=2))
        outst = ctx.enter_context(tc.tile_pool(name="outst", bufs=2))
        ps_tr = ctx.enter_context(
            tc.tile_pool(name="pstr", bufs=2, space=bass.MemorySpace.PSUM)
        )
        ps_mm = ctx.enter_context(
            tc.tile_pool(name="psmm", bufs=6, space=bass.MemorySpace.PSUM)
        )

        # ---- constants ----
        identf_sb = consts.tile([P, P], F32)
        nc.sync.dma_start(identf_sb[:], identf_d[:, :])
        ident_sb = consts.tile([P, P], BF16)
        nc.sync.dma_start(ident_sb[:], ident_d[:, :])
        ones128 = consts.tile([P, P], F32)
        nc.vector.memset(ones128[:], 1.0)

        wparts = stats.tile([P, NT], F32)
        wsum = stats.tile([P, 1], F32)
        gamma = stats.tile([P, 1], F32)
        inv_g = stats.tile([P, 1], F32)
        neg_g = stats.tile([P, 1], F32)
        eta_raw = stats.tile([P, MT], F32)
        eta_all = stats.tile([P, MT], F32)
        inv_eta = stats.tile([P, MT], F32)
        qs_all = stats.tile([P, MT], F32)
        osc_all = stats.tile([P, MT], F32)

        # bias arrives host-prebroadcast [P, N] bf16 (error <= 2e-5, way
        # under the 2e-2 tolerance); one DMA, no on-chip broadcast needed.
        bias_bcast = bias_p.tile([P, N], BF16)
        nc.scalar.dma_start(bias_bcast[:], b_d[:, :])

        # k-major quantized operands
        wqT_all = wqT_p.tile([P, KT * N], BF16)
        wqT_3d = wqT_all[:].rearrange("p (t n) -> p t n", t=KT)
        xqT_all = xqT_p.tile([P, KT * M_CORE], BF16)
        xqT_3d = xqT_all[:].rearrange("p (t m) -> p t m", t=KT)

        # ============ pass 1: stream w, |w|-reduce ============
        x_stage_tiles = {}

        def x_dma(mt, eng):
            t = xstage.tile([P, K], F32, tag="x", name=f"x{mt}")
            eng.dma_start(t[:], x_d[mt * P:(mt + 1) * P, :])
            x_stage_tiles[mt] = t

        x_dma(0, nc.sync)
        x_dma(1, nc.scalar)
        x_dma(2, nc.scalar)

        # pass-1 order is rotated so the LAST WSTAGE_BUFS tiles are 0..3 --
        # column 0's tiles end up resident and need no restream.
        w_resident = {}
        p1_order = list(range(WSTAGE_BUFS, NT)) + list(range(WSTAGE_BUFS))
        for i, nt in enumerate(p1_order):
            t = wstage.tile([P, K], F32, tag="w", name=f"wp1_{nt}")
            eng = nc.sync if i % 2 == 0 else nc.scalar
            eng.dma_start(t[:], w_d[nt * P:(nt + 1) * P, :])
            nc.vector.tensor_reduce(
                wparts[:, nt:nt + 1], t[:], axis=AXIS.X, op=ALU.add,
                apply_absolute_value=True,
            )
            if i >= NT - WSTAGE_BUFS:
                w_resident[nt] = t
        nc.vector.tensor_reduce(wsum[:], wparts[:], axis=AXIS.X, op=ALU.add)

        # ---- x pipeline ----
        def x_head(mt):
            if mt in x_stage_tiles:
                t = x_stage_tiles.pop(mt)
            else:
                t = xstage.tile([P, K], F32, tag="x", name=f"x{mt}")
                eng = nc.sync if mt % 2 == 0 else nc.scalar
                eng.dma_start(t[:], x_d[mt * P:(mt + 1) * P, :])
            nc.vector.tensor_reduce(
                eta_raw[:, mt:mt + 1], t[:], axis=AXIS.X, op=ALU.max,
                apply_absolute_value=True,
            )
            nc.vector.tensor_scalar(
                eta_all[:, mt:mt + 1], eta_raw[:, mt:mt + 1],
                scalar1=1e-5, scalar2=None, op0=ALU.max,
            )
            nc.vector.reciprocal(inv_eta[:, mt:mt + 1], eta_all[:, mt:mt + 1])
            nc.vector.tensor_scalar(
                qs_all[:, mt:mt + 1], inv_eta[:, mt:mt + 1],
                scalar1=127.0, scalar2=None, op0=ALU.mult,
            )
            nc.scalar.activation(
                t[:], t[:], ACTF.Copy, bias=C_MAGIC,
                scale=qs_all[:, mt:mt + 1],
            )
            # -C -> bf16 on ACT (GP elementwise measures ~15us/tile --
            # useless), then cheap bf16 PE transposes + split drains
            q = xqst.tile([P, K], BF16, tag="xq", name=f"xq{mt}")
            nc.scalar.activation(q[:], t[:], ACTF.Copy, bias=-C_MAGIC)
            for g in range(4):
                pt = ps_tr.tile([P, 512], BF16, tag="ptr", name=f"xt{mt}_{g}")
                for j in range(4):
                    kt = g * 4 + j
                    nc.tensor.transpose(
                        pt[:, j * P:(j + 1) * P],
                        q[:, kt * P:(kt + 1) * P],
                        ident_sb[:],
                    )
                dst = xqT_3d[:, g * 4:(g + 1) * 4, mt * P:(mt + 1) * P]
                src = pt[:].rearrange("p (j m) -> p j m", j=4)
                if g % 2 == 0:
                    nc.scalar.copy(dst, src)
                else:
                    nc.vector.tensor_copy(dst, src)

        for mt in range(N_XPRO):
            x_head(mt)

        # ---- gamma epilogue ----
        pg = ps_mm.tile([P, 1], F32, tag="psmm", name="psg")
        nc.tensor.matmul(pg[:], ones128[:, :], wsum[:])
        nc.vector.tensor_scalar(
            gamma[:], pg[:], scalar1=INV_NK, scalar2=1e-5,
            op0=ALU.mult, op1=ALU.max,
        )
        nc.vector.reciprocal(inv_g[:], gamma[:])
        nc.vector.tensor_scalar(
            neg_g[:], gamma[:], scalar1=-1.0, scalar2=None, op0=ALU.mult,
        )

        # ---- w quantize pipeline ----
        def w_restream(nt, eng):
            t = wstage.tile([P, K], F32, tag="w", name=f"wr{nt}")
            eng.dma_start(t[:], w_d[nt * P:(nt + 1) * P, :])
            w_resident[nt] = t

        def w_quant(nt):
            t = w_resident.pop(nt)
            # clip(w, -g, g) on DVE, then *inv_g + C on ACT (rounds on store)
            nc.vector.tensor_scalar(
                t[:], t[:], scalar1=gamma[:, :], scalar2=neg_g[:, :],
                op0=ALU.min, op1=ALU.max,
            )
            nc.scalar.activation(
                t[:], t[:], ACTF.Copy, bias=C_MAGIC, scale=inv_g[:, :]
            )
            for g in range(4):
                pt = ps_tr.tile([P, 512], F32, tag="ptr", name=f"wt{nt}_{g}")
                for j in range(4):
                    kt = g * 4 + j
                    nc.tensor.transpose(
                        pt[:, j * P:(j + 1) * P],
                        t[:, kt * P:(kt + 1) * P],
                        identf_sb[:],
                    )
                dst = wqT_3d[:, g * 4:(g + 1) * 4, nt * P:(nt + 1) * P]
                src = pt[:].rearrange("p (j n) -> p j n", j=4)
                if g % 2 == 0:
                    nc.scalar.activation(dst, src, ACTF.Copy, bias=-C_MAGIC)
                else:
                    nc.vector.tensor_scalar(
                        dst, src, scalar1=C_MAGIC, scalar2=None,
                        op0=ALU.subtract,
                    )

        # tiles 0..3 are resident: quantize immediately after gamma. The
        # restream (tiles 4..15) takes ring priority over x: the quad sweep
        # needs ALL of wqT up front, while x tiles are consumed one m-block
        # (~14.5us) at a time.
        for nt in range(4):
            w_quant(nt)
        ring_plan = (
            [("w", nt) for nt in range(4, NT)]
            + [("x", mt) for mt in range(3, MT)]
        )
        for i, (kind, idx) in enumerate(ring_plan):
            eng = nc.sync if i % 2 == 0 else nc.scalar
            if kind == "w":
                w_restream(idx, eng)
            else:
                x_dma(idx, eng)
        for nt in range(4, NT):
            w_quant(nt)

        # ============ mm: four nb-column sweeps ============
        # A matmul whose lhsT matches the previous one skips the exposed
        # LDWEIGHTS cost (~43ns), so blocks share one lhsT across the nbs
        # of the group.
        def mm_block(mt, nbs):
            if nbs[0] == 0:
                nc.vector.tensor_scalar(
                    osc_all[:, mt:mt + 1], eta_all[:, mt:mt + 1],
                    scalar1=gamma[:, :], scalar2=1.0 / 127.0,
                    op0=ALU.mult, op1=ALU.mult,
                )
            pss = {
                nb: ps_mm.tile([P, 512], F32, tag="psmm", name=f"ps{mt}_{nb}")
                for nb in nbs
            }
            for kt in range(KT):
                lhsT = xqT_3d[:, kt, mt * P:(mt + 1) * P]
                for nb in nbs:
                    nc.tensor.matmul(
                        pss[nb][:],
                        lhsT,
                        wqT_3d[:, kt, nb * 512:(nb + 1) * 512],
                        start=(kt == 0),
                        stop=(kt == KT - 1),
                    )
            for nb in nbs:
                o = outst.tile([P, 512], F32, tag="o", name=f"o{mt}_{nb}")
                nc.vector.scalar_tensor_tensor(
                    o[:], pss[nb][:], osc_all[:, mt:mt + 1],
                    bias_bcast[:, nb * 512:(nb + 1) * 512],
                    op0=ALU.mult, op1=ALU.add,
                )
                eng = nc.sync if nb % 2 == 0 else nc.scalar
                eng.dma_start(
                    out_d[mt * P:(mt + 1) * P, nb * 512:(nb + 1) * 512], o[:]
                )

        # fence + HAM warmup: 16 throwaway matmuls that read tile 15's last
        # wqT slice. Starting the sweep before wqT is complete makes the PE
        # stall mid-block (strict FIFO) and the stall/go pattern re-throttles
        # the clock to 1.2GHz; this holds the sweep until fully fed and
        # enters it with the HAM already at 8/8.
        fence = ps_mm.tile([P, 512], F32, tag="psmm", name="fence")
        for kt in range(KT):
            nc.tensor.matmul(
                fence[:], xqT_3d[:, kt, 0:P], wqT_3d[:, 15, 3 * 512:4 * 512],
                start=(kt == 0), stop=(kt == KT - 1),
            )
        # single quad sweep: all four nb share each lhsT (minimal exposed
        # LDWEIGHTS). The x pipeline interleaves between blocks; x tile mt+3
        # is always ~3 blocks (~44us) ahead of its consumer.
        for mt in range(MT):
            if mt + N_XPRO < MT:
                x_head(mt + N_XPRO)
            mm_block(mt, (0, 1, 2, 3))
    nc.compile()
    return nc


_NC_CACHE = None
LAST_EXEC_NS = None


def _get_nc():
    global _NC_CACHE
    if _NC_CACHE is None:
        _NC_CACHE = _build_program()
    return _NC_CACHE


def _make_in_maps(x, weight, bias):
    xf = np.ascontiguousarray(np.asarray(x, dtype=np.float32).reshape(-1, K))
    w = np.ascontiguousarray(np.asarray(weight, dtype=np.float32))
    b_bf = np.asarray(bias, dtype=np.float32).reshape(1, N).astype(ml_dtypes.bfloat16)
    b = np.ascontiguousarray(np.broadcast_to(b_bf, (P, N)))
    assert xf.shape[0] == N_CORES * M_CORE
    return [
        {
            "x": xf[c * M_CORE:(c + 1) * M_CORE],
            "weight": w,
            "bias": b,
        }
        for c in range(N_CORES)
    ]


def kernel(x, weight, bias):
    global LAST_EXEC_NS
    nc = _get_nc()
    in_maps = _make_in_maps(x, weight, bias)
    trace = bool(int(os.environ.get("BITLINEAR_TRACE", "0")))
    res = run_bass_kernel_spmd(nc, in_maps, list(range(N_CORES)), trace=trace)
    LAST_EXEC_NS = res.exec_time_ns
    out = np.concatenate([res.results[c]["out"] for c in range(N_CORES)], axis=0)
    return out.reshape(np.asarray(x).shape[:-1] + (N,)).astype(np.float32)


# revision 28
# speedup vs baseline: 1.0395x; 1.0395x over previous
"""BitLinear (ternary-weight / int8-activation quantized linear) on 8 trn2 NeuronCores.

Math (matches the jax reference up to fp32 rounding):
    eta   = clip(max|x| along k, 1e-5)             per row
    x_q   = round(x * 127 / eta)    in [-127,127]  (round-half-even)
    gamma = clip(mean|w|, 1e-5)                    scalar
    w_q   = round(clip(w / gamma, -1, 1))          in {-1,0,1}
    out   = (x_q @ w_q^T) * (eta/127 * gamma) + bias

x_q / w_q are small integers exactly representable in bf16 and the PE
accumulates in fp32, so the bf16 matmul is EXACT.  Rounding uses the fp32
magic-number trick  rint(t) = (t + 1.5*2^23) - 1.5*2^23  (round-half-even).
The w clip is applied BEFORE scaling:  round(clip(w/g,-1,1)) ==
round(clip(w,-g,g)/g)  (elements |w|>=g map to +-1 either way), saving a pass.
Both quantized operands are PE-transposed as fp32 magic values (t = q + C);
the PSUM->SBUF drain applies -C and converts to bf16 in the same pass.

Sharding: data-parallel over rows of x (16384 -> 2048 rows/core), weight+bias
replicated.

Per-core schedule: the mm is swept in four nb-COLUMNS (512 out-cols each).
Column nb only depends on w tiles 4nb..4nb+3, so column 0 starts ~12us after
gamma while the rest of w is still being quantized.  LDWEIGHTS is fully
hidden behind the previous matmul's stream on trn2, so the 1-LDW-per-MM cost
of column sweeps is zero.

  pass 1  [0,47us]:  stream all 16 w tiles (2 HWDGE rings), |w|-reduce each
                     (DVE); the last 3 tiles stay resident in their pool bufs.
                     x tiles 0-2 ride at the head of the rings.
  gamma   [~50us]:   PE folds the partial sums; gamma/inv_g/-g scalars.
  W-quant [50..180]: per tile: clip (GP), *inv_g+C (ACT), PE fp32-transpose,
                     drain -C -> bf16 wqT (ACT/DVE), paced so column nb's
                     tiles are ready before its sweep begins.
  X       [50..160]: per tile: abs-max reduce (DVE), *qs+C (ACT), PE
                     fp32-transpose, drain -C -> bf16 xqT (ACT/DVE).
  MM      [~64..]:   4 column sweeps x 16 m-blocks; each block: 16 matmuls
                     (kt-accumulate into one PSUM bank), fused dequant+bias
                     (scalar_tensor_tensor: psum*osc + bias_bcast) -> store.
"""

import os
from contextlib import ExitStack

import numpy as np
import ml_dtypes

import concourse.bass as bass
import concourse.bacc as bacc
import concourse.mybir as mybir
import concourse.tile as tile
from concourse.bass_utils import run_bass_kernel_spmd

P = 128
K = 2048
N = 2048
M_CORE = 2048
KT = K // P          # 16
NT = N // P          # 16
MT = M_CORE // P     # 16
NBLK = N // 512      # 4
N_CORES = 8
C_MAGIC = 12582912.0     # 1.5 * 2**23
INV_NK = 1.0 / (N * K)
N_XPRO = 3           # x tiles processed before the mm loops
WSTAGE_BUFS = 2      # pass-1 w pool; tiles 0..1 (emitted last) stay resident

F32 = mybir.dt.float32
BF16 = mybir.dt.bfloat16
ALU = mybir.AluOpType
AXIS = mybir.AxisListType
ACTF = mybir.ActivationFunctionType


def _build_program():
    nc = bacc.Bacc("TRN2", target_bir_lowering=False, debug=False)

    x_d = nc.dram_tensor("x", [M_CORE, K], F32, kind="ExternalInput").ap()
    w_d = nc.dram_tensor("weight", [N, K], F32, kind="ExternalInput").ap()
    b_d = nc.dram_tensor("bias", [P, N], BF16, kind="ExternalInput").ap()
    out_d = nc.dram_tensor("out", [M_CORE, N], F32, kind="ExternalOutput").ap()
    identf_d = nc.inline_tensor(
        np.eye(P, dtype=np.float32), name="ident128f"
    ).ap()
    ident_d = nc.inline_tensor(
        np.eye(P, dtype=ml_dtypes.bfloat16), name="ident128"
    ).ap()

    with tile.TileContext(nc) as tc, ExitStack() as ctx:
        consts = ctx.enter_context(tc.tile_pool(name="consts", bufs=1))
        stats = ctx.enter_context(tc.tile_pool(name="stats", bufs=1))
        bias_p = ctx.enter_context(tc.tile_pool(name="bias_p", bufs=1))
        wqT_p = ctx.enter_context(tc.tile_pool(name="wqT", bufs=1))
        xqT_p = ctx.enter_context(tc.tile_pool(name="xqT", bufs=1))
        wstage = ctx.enter_context(tc.tile_pool(name="wstage", bufs=WSTAGE_BUFS))
        wrst = ctx.enter_context(tc.tile_pool(name="wrst", bufs=2))
        xstage = ctx.enter_context(tc.tile_pool(name="xstage", bufs=3))
        xqst = ctx.enter_context(tc.tile_pool(name="xqst", bufs=2))
        outst = ctx.enter_context(tc.tile_pool(name="outst", bufs=2))
        ps_tr = ctx.enter_context(
            tc.tile_pool(name="pstr", bufs=2, space=bass.MemorySpace.PSUM)
        )
        ps_mm = ctx.enter_context(
            tc.tile_pool(name="psmm", bufs=6, space=bass.MemorySpace.PSUM)
        )

        # ---- constants ----
        identf_sb = consts.tile([P, P], F32)
        nc.sync.dma_start(identf_sb[:], identf_d[:, :])
        ident_sb = consts.tile([P, P], BF16)
        nc.sync.dma_start(ident_sb[:], ident_d[:, :])
        ones128 = consts.tile([P, P], F32)
        nc.vector.memset(ones128[:], 1.0)

        wparts = stats.tile([P, NT], F32)
        wsum = stats.tile([P, 1], F32)
        gamma = stats.tile([P, 1], F32)
        inv_g = stats.tile([P, 1], F32)
        neg_g = stats.tile([P, 1], F32)
        eta_raw = stats.tile([P, MT], F32)
        eta_all = stats.tile([P, MT], F32)
        inv_eta = stats.tile([P, MT], F32)
        qs_all = stats.tile([P, MT], F32)
        osc_all = stats.tile([P, MT], F32)

        # bias arrives host-prebroadcast [P, N] bf16 (error <= 2e-5, way
        # under the 2e-2 tolerance); one DMA, no on-chip broadcast needed.
        bias_bcast = bias_p.tile([P, N], BF16)
        nc.scalar.dma_start(bias_bcast[:], b_d[:, :])

        # k-major quantized operands
        wqT_all = wqT_p.tile([P, KT * N], BF16)
        wqT_3d = wqT_all[:].rearrange("p (t n) -> p t n", t=KT)
        xqT_all = xqT_p.tile([P, KT * M_CORE], BF16)
        xqT_3d = xqT_all[:].rearrange("p (t m) -> p t m", t=KT)

        # ============ pass 1: stream w, |w|-reduce ============
        x_stage_tiles = {}

        def x_dma(mt, eng):
            t = xstage.tile([P, K], F32, tag="x", name=f"x{mt}")
            eng.dma_start(t[:], x_d[mt * P:(mt + 1) * P, :])
            x_stage_tiles[mt] = t

        x_dma(0, nc.sync)
        x_dma(1, nc.scalar)
        x_dma(2, nc.scalar)

        # pass-1 order is rotated so the LAST WSTAGE_BUFS tiles are 0..3 --
        # column 0's tiles end up resident and need no restream.
        w_resident = {}
        p1_order = list(range(WSTAGE_BUFS, NT)) + list(range(WSTAGE_BUFS))
        for i, nt in enumerate(p1_order):
            t = wstage.tile([P, K], F32, tag="w", name=f"wp1_{nt}")
            eng = nc.sync if i % 2 == 0 else nc.scalar
            eng.dma_start(t[:], w_d[nt * P:(nt + 1) * P, :])
            nc.vector.tensor_reduce(
                wparts[:, nt:nt + 1], t[:], axis=AXIS.X, op=ALU.add,
                apply_absolute_value=True,
            )
            if i >= NT - WSTAGE_BUFS:
                w_resident[nt] = t
        nc.vector.tensor_reduce(wsum[:], wparts[:], axis=AXIS.X, op=ALU.add)

        # ---- x pipeline ----
        def x_head(mt):
            if mt in x_stage_tiles:
                t = x_stage_tiles.pop(mt)
            else:
                t = xstage.tile([P, K], F32, tag="x", name=f"x{mt}")
                eng = nc.sync if mt % 2 == 0 else nc.scalar
                eng.dma_start(t[:], x_d[mt * P:(mt + 1) * P, :])
            nc.vector.tensor_reduce(
                eta_raw[:, mt:mt + 1], t[:], axis=AXIS.X, op=ALU.max,
                apply_absolute_value=True,
            )
            nc.vector.tensor_scalar(
                eta_all[:, mt:mt + 1], eta_raw[:, mt:mt + 1],
                scalar1=1e-5, scalar2=None, op0=ALU.max,
            )
            nc.vector.reciprocal(inv_eta[:, mt:mt + 1], eta_all[:, mt:mt + 1])
            nc.vector.tensor_scalar(
                qs_all[:, mt:mt + 1], inv_eta[:, mt:mt + 1],
                scalar1=127.0, scalar2=None, op0=ALU.mult,
            )
            nc.scalar.activation(
                t[:], t[:], ACTF.Copy, bias=C_MAGIC,
                scale=qs_all[:, mt:mt + 1],
            )
            # -C -> bf16 on ACT (GP elementwise measures ~15us/tile --
            # useless), then cheap bf16 PE transposes + split drains
            q = xqst.tile([P, K], BF16, tag="xq", name=f"xq{mt}")
            nc.scalar.activation(q[:], t[:], ACTF.Copy, bias=-C_MAGIC)
            for g in range(4):
                pt = ps_tr.tile([P, 512], BF16, tag="ptr", name=f"xt{mt}_{g}")
                for j in range(4):
                    kt = g * 4 + j
                    nc.tensor.transpose(
                        pt[:, j * P:(j + 1) * P],
                        q[:, kt * P:(kt + 1) * P],
                        ident_sb[:],
                    )
                dst = xqT_3d[:, g * 4:(g + 1) * 4, mt * P:(mt + 1) * P]
                src = pt[:].rearrange("p (j m) -> p j m", j=4)
                if g % 2 == 0:
                    nc.scalar.copy(dst, src)
                else:
                    nc.vector.tensor_copy(dst, src)

        # ---- gamma epilogue ----
        pg = ps_mm.tile([P, 1], F32, tag="psmm", name="psg")
        nc.tensor.matmul(pg[:], ones128[:, :], wsum[:])
        nc.vector.tensor_scalar(
            gamma[:], pg[:], scalar1=INV_NK, scalar2=1e-5,
            op0=ALU.mult, op1=ALU.max,
        )
        nc.vector.reciprocal(inv_g[:], gamma[:])
        nc.vector.tensor_scalar(
            neg_g[:], gamma[:], scalar1=-1.0, scalar2=None, op0=ALU.mult,
        )

        # x0..2 processing is emitted after the gamma chain so their DVE
        # reduces don't delay gamma (their data is preloaded anyway).
        for mt in range(N_XPRO):
            x_head(mt)

        # ---- w quantize pipeline ----
        def w_restream(nt, eng):
            t = wrst.tile([P, K], F32, tag="wr", name=f"wr{nt}")
            eng.dma_start(t[:], w_d[nt * P:(nt + 1) * P, :])
            w_resident[nt] = t

        def w_quant(nt):
            t = w_resident.pop(nt)
            # clip(w, -g, g) on DVE, then *inv_g + C on ACT (rounds on store)
            nc.vector.tensor_scalar(
                t[:], t[:], scalar1=gamma[:, :], scalar2=neg_g[:, :],
                op0=ALU.min, op1=ALU.max,
            )
            nc.scalar.activation(
                t[:], t[:], ACTF.Copy, bias=C_MAGIC, scale=inv_g[:, :]
            )
            for g in range(4):
                pt = ps_tr.tile([P, 512], F32, tag="ptr", name=f"wt{nt}_{g}")
                for j in range(4):
                    kt = g * 4 + j
                    nc.tensor.transpose(
                        pt[:, j * P:(j + 1) * P],
                        t[:, kt * P:(kt + 1) * P],
                        identf_sb[:],
                    )
                dst = wqT_3d[:, g * 4:(g + 1) * 4, nt * P:(nt + 1) * P]
                src = pt[:].rearrange("p (j n) -> p j n", j=4)
                if g % 2 == 0:
                    nc.scalar.activation(dst, src, ACTF.Copy, bias=-C_MAGIC)
                else:
                    nc.vector.tensor_scalar(
                        dst, src, scalar1=C_MAGIC, scalar2=None,
                        op0=ALU.subtract,
                    )

        # tiles 0..1 are resident: quantize immediately after gamma. The
        # restream rides a dedicated 2-buf pool so tiles 2..5 transfer before
        # gamma even lands; phase A needs tiles 0..7, phase B 8..15.
        for nt in range(2):
            w_quant(nt)
        ring_plan = (
            [("w", nt) for nt in range(2, 8)]
            + [("x", mt) for mt in range(3, 6)]
            + [("w", nt) for nt in range(8, 10)]
            + [("x", mt) for mt in range(6, MT)]
            + [("w", nt) for nt in range(10, NT)]
        )
        for i, (kind, idx) in enumerate(ring_plan):
            eng = nc.sync if i % 2 == 0 else nc.scalar
            if kind == "w":
                w_restream(idx, eng)
            else:
                x_dma(idx, eng)
        for nt in range(2, 8):
            w_quant(nt)

        # ============ mm: four nb-column sweeps ============
        # A matmul whose lhsT matches the previous one skips the exposed
        # LDWEIGHTS cost (~43ns), so blocks share one lhsT across the nbs
        # of the group.
        def mm_block(mt, nbs):
            if nbs[0] == 0:
                nc.vector.tensor_scalar(
                    osc_all[:, mt:mt + 1], eta_all[:, mt:mt + 1],
                    scalar1=gamma[:, :], scalar2=1.0 / 127.0,
                    op0=ALU.mult, op1=ALU.mult,
                )
            pss = {
                nb: ps_mm.tile([P, 512], F32, tag="psmm", name=f"ps{mt}_{nb}")
                for nb in nbs
            }
            for kt in range(KT):
                lhsT = xqT_3d[:, kt, mt * P:(mt + 1) * P]
                for nb in nbs:
                    nc.tensor.matmul(
                        pss[nb][:],
                        lhsT,
                        wqT_3d[:, kt, nb * 512:(nb + 1) * 512],
                        start=(kt == 0),
                        stop=(kt == KT - 1),
                    )
            for nb in nbs:
                o = outst.tile([P, 512], F32, tag="o", name=f"o{mt}_{nb}")
                nc.vector.scalar_tensor_tensor(
                    o[:], pss[nb][:], osc_all[:, mt:mt + 1],
                    bias_bcast[:, nb * 512:(nb + 1) * 512],
                    op0=ALU.mult, op1=ALU.add,
                )
                nc.sync.dma_start(
                    out_d[mt * P:(mt + 1) * P, nb * 512:(nb + 1) * 512], o[:]
                )

        # phase A — columns 0+1 paired (lhsT shared); phase-B quants and the
        # x pipeline interleave between blocks.
        for mt in range(MT):
            if mt + N_XPRO < MT:
                x_head(mt + N_XPRO)
            if mt % 2 == 0:
                w_quant(8 + mt // 2)    # tiles 8..15 for phase B
            mm_block(mt, (0, 1))
        # phase B — columns 2+3 paired
        for mt in range(MT):
            mm_block(mt, (2, 3))
    nc.compile()
    return nc


_NC_CACHE = None
LAST_EXEC_NS = None


def _get_nc():
    global _NC_CACHE
    if _NC_CACHE is None:
        _NC_CACHE = _build_program()
    return _NC_CACHE


def _make_in_maps(x, weight, bias):
    xf = np.ascontiguousarray(np.asarray(x, dtype=np.float32).reshape(-1, K))
    w = np.ascontiguousarray(np.asarray(weight, dtype=np.float32))
    b_bf = np.asarray(bias, dtype=np.float32).reshape(1, N).astype(ml_dtypes.bfloat16)
    b = np.ascontiguousarray(np.broadcast_to(b_bf, (P, N)))
    assert xf.shape[0] == N_CORES * M_CORE
    return [
        {
            "x": xf[c * M_CORE:(c + 1) * M_CORE],
            "weight": w,
            "bias": b,
        }
        for c in range(N_CORES)
    ]


def kernel(x, weight, bias):
    global LAST_EXEC_NS
    nc = _get_nc()
    in_maps = _make_in_maps(x, weight, bias)
    trace = bool(int(os.environ.get("BITLINEAR_TRACE", "0")))
    res = run_bass_kernel_spmd(nc, in_maps, list(range(N_CORES)), trace=trace)
    LAST_EXEC_NS = res.exec_time_ns
    out = np.concatenate([res.results[c]["out"] for c in range(N_CORES)], axis=0)
    return out.reshape(np.asarray(x).shape[:-1] + (N,)).astype(np.float32)
